# revision 24
# baseline (speedup 1.0000x reference)
"""CharRNN Trainium2 kernel.

Computes, for x:[B,L] int tokens:
    xh    = embedding[x] @ W_xh + b_h          (folded: E2H = embedding @ W_xh)
    h_t   = tanh(xh_t + h_{t-1} @ W_hh)        (sequential scan over L)
    logits= h @ W_hy + b_y
returns (logits:[B,L,V] f32, final_hidden:[B,H] f32).

Strategy (8 NeuronCores, data-parallel over batch, 64 rows/core; all f32):
  * h lives transposed in SBUF as [H=128 partitions, 64 batch].
  * xh is injected into PSUM by a one-hot matmul (E2H stationary, one-hot of
    the tokens as moving operand, start=True); the recurrence matmul
    (W_hh stationary, h moving) accumulates on top with start=False; the
    tanh reads PSUM and writes h back to SBUF with b_h as the free
    per-partition ACT bias.  One wide 64-lane chain per core: the measured
    HW round-trip of the dependent MM->tanh->MM loop (~150 ns) is shorter
    than the cost of splitting the batch into staggered sub-chains (extra
    weight reloads on PE dominate).
  * Logits: one chunk (16 steps) at a time, W_hy stationary, the h-history
    as the 512-wide moving operand -> PSUM [V, t*b]; b_y added as a
    per-partition DVE scalar; DMA'd to a dense scratch layout that the
    host permutes into [B,L,V].
"""

import threading

import numpy as np

B, L, V, E, H = 512, 1024, 32, 32, 128
NCORES = 8
BL = B // NCORES  # 64 batch rows per core
CH = 1  # sub-chains per core (HW round-trip is short; 1 wide chain wins)
BC = BL // CH  # 32 batch rows per sub-chain
TC = 16  # scan steps per chunk (chunk of xh = 2 PSUM banks)
NCHUNK = L // TC
JG = 2  # steps per logits matmul group (JG*BL <= 128 PSUM partitions)
F32 = np.float32

_lock = threading.Lock()
_cache = {}

# structural knobs (sim-searched)
CONFIG = {
    "oh_split": 1,   # one-hot matmuls per chain-chunk
    "jg": JG,        # steps per logits matmul group
    "hh_bufs": 2,
    "xh_bufs": 2,
    "lg_bufs": 2,
    "oh_bufs": 3,
    "ch": CH,
    "no_logits": False,   # diag: skip logits path
    "no_rec": False,      # diag: skip recurrence matmul (breaks numerics)
    "lraw_internal": False,  # timing: keep lraw on-device (not an output)
    "f32r_oh": False,     # one-hot matmul in float32r (1 cyc/row at N=512)
    "lg_vmajor": "f32",   # logits: W_hy stationary, out psum [V, (t,b)]
    "rec_f32r": False,    # whole recurrence in float32r (faster MM, ~3e-4 err)
}


def _build_program(reps=1, mode="f32"):
    import concourse.bacc as bacc
    import concourse.bass as bass
    import concourse.tile as tile
    from concourse import mybir

    f32 = mybir.dt.float32
    wdt = mybir.dt.bfloat16 if mode == "bf16" else mybir.dt.float32
    Tanh = mybir.ActivationFunctionType.Tanh
    jg = CONFIG["jg"]
    oh_split = CONFIG["oh_split"]
    ch = CONFIG["ch"]
    bc = BL // ch
    rec_r = bool(CONFIG["rec_f32r"]) and mode != "bf16"

    nc = bacc.Bacc("TRN2", target_bir_lowering=False, debug=False,
                   num_devices=NCORES)

    xTf = nc.dram_tensor("xTf", [L, BL], f32, kind="ExternalInput").ap()
    whh_d = nc.dram_tensor("Whh", [H, H], wdt, kind="ExternalInput").ap()
    e2h_d = nc.dram_tensor("E2H", [V, H], wdt, kind="ExternalInput").ap()
    why_d = nc.dram_tensor("Why", [H, V], wdt, kind="ExternalInput").ap()
    bh_d = nc.dram_tensor("bh", [H, 1], f32, kind="ExternalInput").ap()
    byr_d = nc.dram_tensor("byr", [1, TC // CONFIG["jg"], V], f32, kind="ExternalInput").ap()
    iota_d = nc.dram_tensor("iota", [V, 1], f32, kind="ExternalInput").ap()
    by_d = nc.dram_tensor("by", [V, 1], f32, kind="ExternalInput").ap()

    lraw_kind = "Internal" if CONFIG["lraw_internal"] else "ExternalOutput"
    if CONFIG["lg_vmajor"]:
        lraw = nc.dram_tensor("lraw", [NCHUNK, ch, V, TC, bc], f32,
                              kind=lraw_kind).ap()
    else:
        lraw = nc.dram_tensor("lraw", [NCHUNK, ch, jg * bc, TC // jg, V], f32,
                              kind=lraw_kind).ap()
    hT_dt = (mybir.dt.float32r if (CONFIG["rec_f32r"] and mode != "bf16")
             else wdt)
    hT_d = nc.dram_tensor("hT", [H, BL], hT_dt, kind="ExternalOutput").ap()

    with tile.TileContext(nc) as tc:
        with (
            tc.tile_pool(name="const", bufs=1) as const,
            tc.tile_pool(name="xin", bufs=CONFIG["oh_bufs"]) as xpool,
            tc.tile_pool(name="oh", bufs=CONFIG["oh_bufs"]) as ohpool,
            tc.tile_pool(name="hh", bufs=CONFIG["hh_bufs"]) as hpool,
            tc.tile_pool(name="lsb", bufs=3) as lpool,
            tc.tile_pool(name="xh_ps", bufs=CONFIG["xh_bufs"], space="PSUM") as xh_ps,
            tc.tile_pool(name="lg_ps", bufs=CONFIG["lg_bufs"], space="PSUM") as lg_ps,
        ):
            whh_sb = const.tile([H, H], wdt)
            nc.sync.dma_start(out=whh_sb, in_=whh_d)
            e2h_sb = const.tile([V, H], wdt)
            nc.sync.dma_start(out=e2h_sb, in_=e2h_d)
            why_sb = const.tile([H, V], wdt)
            nc.sync.dma_start(out=why_sb, in_=why_d)
            bh_sb = const.tile([H, 1], f32)
            nc.sync.dma_start(out=bh_sb, in_=bh_d)
            iota_sb = const.tile([V, 1], f32)
            nc.sync.dma_start(out=iota_sb, in_=iota_d)
            by_sb = const.tile([V, 1], f32)
            nc.sync.dma_start(out=by_sb, in_=by_d)
            byr_sb = const.tile([jg * bc, TC // jg, V], f32)
            nc.sync.dma_start(
                out=byr_sb,
                in_=bass.AP(tensor=byr_d.tensor, offset=byr_d.offset,
                            ap=[[0, jg * bc]] + list(byr_d.ap[1:])),
            )
            f32r = mybir.dt.float32r
            hdt = f32r if rec_r else wdt
            if rec_r:
                whh_r = const.tile([H, H], f32r)
                nc.vector.tensor_copy(whh_r[:], whh_sb[:])
            if CONFIG["f32r_oh"] and mode != "bf16":
                e2h_r = const.tile([V, H], f32r)
                nc.vector.tensor_copy(e2h_r[:], e2h_sb[:])
            if CONFIG["lg_vmajor"] and (CONFIG["lg_vmajor"] != "f32" or rec_r):
                why_r = const.tile([H, V], f32r)
                nc.vector.tensor_copy(why_r[:], why_sb[:])
            if rec_r:
                h0f = const.tile([H, bc], f32)
                nc.vector.memset(h0f, 0.0)
                h0 = const.tile([H, bc], hdt)
                nc.vector.tensor_copy(h0[:], h0f[:])
            else:
                h0 = const.tile([H, bc], hdt)
                nc.vector.memset(h0, 0.0)

            hh = [None] * ch

            for rep in range(reps):
              h_prev = [h0[:]] * ch
              for c in range(NCHUNK):
                # token slice, replicated across the V=32 partitions
                xb = xpool.tile([V, TC, BL], f32, tag="xb")
                src = bass.AP(
                    tensor=xTf.tensor,
                    offset=xTf.offset + c * TC * BL,
                    ap=[[0, V], [BL, TC], [1, BL]],
                )
                nc.sync.dma_start(out=xb, in_=src)

                # one-hot: oh[v, t, b] = (x[t,b] == v)
                oh_dt = f32r if (CONFIG["f32r_oh"] and mode != "bf16") else wdt
                oh = ohpool.tile([V, TC, BL], oh_dt, tag="oh")
                nc.vector.tensor_scalar(
                    out=oh[:], in0=xb[:], scalar1=iota_sb[:, 0:1], scalar2=None,
                    op0=mybir.AluOpType.is_equal,
                )

                # xh chunk into PSUM, one bank per sub-chain:
                # xh_X[h, t, b] = E2H.T @ onehot[:, :, chainX]
                xh = [
                    xh_ps.tile([H, TC, bc], f32, tag=f"xh{X}", name=f"xh{X}")
                    for X in range(ch)
                ]
                tsub = min(TC, 512 // bc)  # steps per PSUM bank
                osub = max(1, tsub // oh_split)  # steps per one-hot matmul
                lhs_oh_full = (e2h_r if (CONFIG["f32r_oh"] and mode != "bf16")
                               else e2h_sb)
                for X in range(ch):
                    for o in range(TC // osub):
                        nc.tensor.matmul(
                            xh[X][:, o * osub:(o + 1) * osub, :],
                            lhs_oh_full[:],
                            oh[:, o * osub:(o + 1) * osub,
                               X * bc:(X + 1) * bc],
                            start=bool((o * osub) % tsub == 0), stop=False,
                            skip_group_check=True)
                    hh[X] = hpool.tile([H, TC, bc], hdt, tag=f"hh{X}",
                                       name=f"hh{X}")

                for t in range(TC):
                    for X in range(ch):
                        sl = xh[X][:, t, :]
                        if not CONFIG["no_rec"]:
                            nc.tensor.matmul(
                                sl, whh_r[:] if rec_r else whh_sb[:],
                                h_prev[X],
                                start=False,
                                stop=bool(t % tsub == tsub - 1),
                                skip_group_check=True,
                            )
                        nc.scalar.activation(
                            out=hh[X][:, t, :], in_=sl, func=Tanh,
                            bias=bh_sb[:, 0:1], scale=1.0,
                        )
                        h_prev[X] = hh[X][:, t, :]

                # logits (v-major): W_hy stationary in f32r, h moving;
                # out psum [V, steps*batch], bias via per-partition scalar add
                if CONFIG["lg_vmajor"] and not CONFIG["no_logits"]:
                    tsub_l = 512 // bc
                    use_r = CONFIG["lg_vmajor"] != "f32" and not rec_r
                    for X in range(ch):
                        if rec_r:
                            hmov = hh[X]
                            wmov = why_r
                        elif use_r:
                            hmov = lpool.tile([H, TC, bc], f32r, tag="hhr",
                                              name="hhr")
                            nc.vector.tensor_copy(hmov[:], hh[X][:])
                            wmov = why_r
                        else:
                            hmov = hh[X]
                            wmov = why_sb
                        lgv = lg_ps.tile([V, TC, bc], f32, tag="lgv",
                                         name="lgv")
                        for o in range(TC // tsub_l):
                            nc.tensor.matmul(
                                lgv[:, o * tsub_l:(o + 1) * tsub_l, :],
                                wmov[:],
                                hmov[:, o * tsub_l:(o + 1) * tsub_l, :],
                                start=True, stop=True, skip_group_check=True)
                        lsbv = lpool.tile([V, TC, bc], f32, tag="lsbv",
                                          name="lsbv")
                        nc.vector.tensor_scalar(
                            out=lsbv[:], in0=lgv[:], scalar1=by_sb[:, 0:1],
                            scalar2=None, op0=mybir.AluOpType.add)
                        nc.gpsimd.dma_start(out=lraw[c, X], in_=lsbv[:])
                # logits: jg steps per matmul; psum rows = (step, batch)
                for X in range(ch if (not CONFIG["no_logits"]
                                      and not CONFIG["lg_vmajor"]) else 0):
                    lg = lg_ps.tile([jg * bc, TC // jg, V], f32, tag="lg",
                                    name="lg")
                    for j in range(TC // jg):
                        nc.tensor.matmul(
                            lg[:, j, :], hh[X][:, j * jg:(j + 1) * jg, :],
                            why_sb[:], start=True, stop=True,
                            skip_group_check=True,
                        )
                    lsb = lpool.tile([jg * bc, TC // jg, V], f32, tag="lsb",
                                     name="lsb")
                    nc.vector.tensor_add(lsb[:], lg[:], byr_sb[:])
                    nc.gpsimd.dma_start(out=lraw[c, X], in_=lsb[:])

            for X in range(ch):
                nc.gpsimd.dma_start(out=hT_d[:, X * bc:(X + 1) * bc],
                                    in_=h_prev[X])

    nc.compile()
    return nc


MODE = "f32"


def _get_program(reps=1, mode=None):
    if mode is None:
        mode = MODE
    with _lock:
        key = ("nc", reps, mode, tuple(sorted(CONFIG.items())))
        if key not in _cache:
            _cache[key] = _build_program(reps, mode)
        return _cache[key]


def _host_prep(x, embedding, W_xh, W_hh, b_h, W_hy, b_y, mode=None):
    if mode is None:
        mode = MODE
    import ml_dtypes
    wnp = ml_dtypes.bfloat16 if mode == "bf16" else np.float32
    x = np.asarray(x)
    emb = np.asarray(embedding, dtype=F32)
    W_xh = np.asarray(W_xh, dtype=F32)
    W_hh = np.ascontiguousarray(np.asarray(W_hh, dtype=F32))
    b_h = np.asarray(b_h, dtype=F32)
    W_hy = np.ascontiguousarray(np.asarray(W_hy, dtype=F32))
    b_y = np.asarray(b_y, dtype=F32)

    e2h = np.ascontiguousarray((emb @ W_xh).astype(wnp))  # [V, H]
    W_hh = np.ascontiguousarray(W_hh.astype(wnp))
    W_hy = np.ascontiguousarray(W_hy.astype(wnp))
    xT = x.T.astype(F32)  # [L, B]
    jg = CONFIG["jg"]
    byr = np.ascontiguousarray(
        np.broadcast_to(b_y, (TC // jg, V)).reshape(1, TC // jg, V))
    iota = np.arange(V, dtype=F32).reshape(V, 1)
    bh = np.ascontiguousarray(b_h.reshape(H, 1))

    in_maps = []
    for i in range(NCORES):
        in_maps.append({
            "xTf": np.ascontiguousarray(xT[:, i * BL:(i + 1) * BL]),
            "Whh": W_hh,
            "E2H": e2h,
            "Why": W_hy,
            "bh": bh,
            "byr": byr,
            "iota": iota,
            "by": np.ascontiguousarray(b_y.reshape(V, 1)),
        })
    return in_maps


def _assemble(results):
    logits = np.empty((B, L, V), dtype=F32)
    final_hidden = np.empty((B, H), dtype=F32)
    for i, res in enumerate(results):
        jg = CONFIG["jg"]
        ch = CONFIG["ch"]
        bc = BL // ch
        lraw = res["lraw"]
        if CONFIG["lg_vmajor"]:
            # [c, X, v, t, b] -> [X, b, c, t, v]
            blk = lraw.transpose(1, 4, 0, 3, 2).reshape(BL, L, V)
        else:
            blk = lraw.reshape(NCHUNK, ch, jg, bc, TC // jg, V)
            # [c, X, s, b, j, v] -> [X, b, c, j, s, v]
            blk = blk.transpose(1, 3, 0, 4, 2, 5).reshape(BL, L, V)
        logits[i * BL:(i + 1) * BL] = blk
        final_hidden[i * BL:(i + 1) * BL] = res["hT"].T.astype(F32)
    return logits, final_hidden


def kernel(**inputs):
    from concourse.bass_utils import run_bass_kernel_spmd

    nc = _get_program()
    in_maps = _host_prep(**inputs)
    out = run_bass_kernel_spmd(nc, in_maps, core_ids=list(range(NCORES)))
    return _assemble(out.results)


if __name__ == "__main__":
    rng = np.random.default_rng(0)
    ins = {
        "x": rng.integers(0, V, size=(B, L)),
        "embedding": rng.standard_normal((V, E)).astype(F32),
        "W_xh": (rng.standard_normal((E, H)) * 0.01).astype(F32),
        "W_hh": (rng.standard_normal((H, H)) * 0.01).astype(F32),
        "b_h": np.zeros(H, dtype=F32),
        "W_hy": (rng.standard_normal((H, V)) * 0.01).astype(F32),
        "b_y": np.zeros(V, dtype=F32),
    }
    logits, fh = kernel(**ins)
    print(logits.shape, fh.shape)
